# revision 8
# baseline (speedup 1.0000x reference)
"""Trainium2 Bass kernel for nn_CentroidLoss (B=8, H=W=512, L=64, sigma=2).

Self-contained: hardcodes shapes/sharding. Shards the batch dim across 8
NeuronCores (one image per core); each core computes its partial sum of
squared errors; host sums and divides for the mean.
"""
import numpy as np

import concourse.bacc as bacc
import concourse.bass as bass
import concourse.tile as tile
from concourse import mybir
from concourse.bass_utils import run_bass_kernel_spmd

H = 512
W = 512
L = 64
SIGMA = 2.0
BIG = np.float32(1e30)
NCORES = 8
NYC = H // 128          # 4 row-chunks of 128 partitions
NXC = W // 8            # 64 col-chunks of 8 columns
F = 8 * L               # 512 free elems per chunk (dx-major, l-minor)

DT = mybir.dt.float32


def _constants():
    """Host-precomputed constant inputs (pure functions of shapes)."""
    p = np.arange(128, dtype=np.float32)
    x = np.arange(W, dtype=np.float32)

    # l-pattern repeated along free dim: value = f % 64, same every partition
    iotaL = np.tile(np.arange(L, dtype=np.float32), 8)[None, :].repeat(128, 0)

    # Phase-1 lhsT variants: for (yc, xc): cols (1, y, xb)
    selAll = np.zeros((128, NYC * NXC * 3), dtype=np.float32)
    for yc in range(NYC):
        for xc in range(NXC):
            b = (yc * NXC + xc) * 3
            selAll[:, b + 0] = 1.0
            selAll[:, b + 1] = 128 * yc + p
            selAll[:, b + 2] = 8 * xc

    # T1 lhsT: rows (x^2, x, 1) for x = 128*c + p
    T1lhs = np.stack([x * x, x, np.ones_like(x)]).astype(np.float32)  # [3, 512]

    # Phase-2 lhsT per yc: rows (y, 1)
    Y2 = np.zeros((2, NYC * 128), dtype=np.float32)
    for yc in range(NYC):
        Y2[0, 128 * yc : 128 * (yc + 1)] = 128 * yc + p
        Y2[1, 128 * yc : 128 * (yc + 1)] = 1.0

    # ACT bias per yc: -(y^2)/(2 sigma^2)
    negy2 = np.zeros((128, NYC), dtype=np.float32)
    for yc in range(NYC):
        negy2[:, yc] = -((128 * yc + p) ** 2) / (2.0 * SIGMA * SIGMA)

    # dx value at free pos (dx*64 + l)
    dxpat = np.repeat(np.arange(8, dtype=np.float32), L)[None, :]

    # background-label mask: 0 at l=0, 1 elsewhere
    lmask = np.ones((1, L), dtype=np.float32)
    lmask[0, 0] = 0.0

    return dict(iotaL=iotaL, selAll=selAll, T1lhs=T1lhs, Y2=Y2,
                negy2=negy2, dxpat=dxpat, lmask=lmask)


def build(nc):
    """Declare IO and emit the kernel into nc (inside a TileContext)."""
    lab_d = nc.dram_tensor("lab", [H, W], mybir.dt.int32, kind="ExternalInput")
    pred_d = nc.dram_tensor("pred", [H, W], DT, kind="ExternalInput")
    iotaL_d = nc.dram_tensor("iotaL", [128, F], DT, kind="ExternalInput")
    selAll_d = nc.dram_tensor("selAll", [128, NYC * NXC * 3], DT, kind="ExternalInput")
    T1lhs_d = nc.dram_tensor("T1lhs", [3, W], DT, kind="ExternalInput")
    Y2_d = nc.dram_tensor("Y2", [2, NYC * 128], DT, kind="ExternalInput")
    negy2_d = nc.dram_tensor("negy2", [128, NYC], DT, kind="ExternalInput")
    dxpat_d = nc.dram_tensor("dxpat", [1, F], DT, kind="ExternalInput")
    lmask_d = nc.dram_tensor("lmask", [1, L], DT, kind="ExternalInput")
    out_d = nc.dram_tensor("out", [1, 1], DT, kind="ExternalOutput")

    rs1_d = nc.dram_tensor("rs1scratch", [4, 128, L], DT)   # T1 flatten bounce
    rs0_d = nc.dram_tensor("rs0scratch", [128, L], DT)      # -2cy flatten bounce

    AL = mybir.AluOpType
    with tile.TileContext(nc) as tc:
        with (
            tc.tile_pool(name="const", bufs=1) as cpool,
            tc.tile_pool(name="work", bufs=3) as wpool,
            tc.tile_pool(name="small", bufs=1) as spool,
            tc.tile_pool(name="ps1", bufs=1, space="PSUM") as ps1,
            tc.tile_pool(name="ps2", bufs=2, space="PSUM") as ps2,
        ):
            iotaL = cpool.tile([128, F], DT)
            selAll = cpool.tile([128, NYC * NXC * 3], DT)
            T1lhs = cpool.tile([3, W], DT)
            Y2 = cpool.tile([2, NYC * 128], DT)
            negy2 = cpool.tile([128, NYC], DT)
            dxpat = cpool.tile([1, F], DT)
            lmask = cpool.tile([1, L], DT)
            for t, d in ((iotaL, iotaL_d), (selAll, selAll_d), (T1lhs, T1lhs_d),
                         (Y2, Y2_d), (negy2, negy2_d), (dxpat, dxpat_d),
                         (lmask, lmask_d)):
                nc.sync.dma_start(t[:], d.ap())

            # ---- Phase 1: label histogram (cnt, sy, sx) ----
            lab_i = cpool.tile([128, NYC * W], mybir.dt.int32)
            nc.sync.dma_start(
                lab_i[:].rearrange("p (yc x) -> p yc x", x=W),
                lab_d.ap().rearrange("(yc p) x -> p yc x", p=128))
            lab_f = cpool.tile([128, NYC * W], DT)
            nc.vector.tensor_copy(lab_f[:], lab_i[:])

            psum1 = ps1.tile([3, F], DT)
            n = 0
            for yc in range(NYC):
                for xc in range(NXC):
                    oh = wpool.tile([128, F], DT, tag="onehot")
                    lab_b = (lab_f[:, yc * W + xc * 8 : yc * W + xc * 8 + 8]
                             .unsqueeze(-1).broadcast_to([128, 8, L]))
                    nc.vector.tensor_tensor(
                        oh[:].rearrange("p (dx l) -> p dx l", l=L),
                        iotaL[:].rearrange("p (dx l) -> p dx l", l=L),
                        lab_b, AL.is_equal)
                    b = (yc * NXC + xc) * 3
                    nc.tensor.matmul(
                        psum1[:], selAll[:, b : b + 3], oh[:],
                        start=(n == 0), stop=(n == NYC * NXC - 1))
                    n += 1

            # ---- Phase 1 fold: [3, F] -> cnt/sy/sx [1, 64] ----
            S1 = spool.tile([3, F], DT)
            nc.vector.tensor_copy(S1[:], psum1[:])
            # DVE ops need base partition 0 -> extract rows via DMA
            row_c = spool.tile([1, F], DT)
            row_y = spool.tile([1, F], DT)
            row_x = spool.tile([1, F], DT)
            nc.sync.dma_start(row_c[:], S1[0:1, :])
            nc.sync.dma_start(row_y[:], S1[1:2, :])
            nc.sync.dma_start(row_x[:], S1[2:3, :])

            cnt = spool.tile([1, L], DT)
            sy = spool.tile([1, L], DT)
            sx = spool.tile([1, L], DT)
            tmpF = spool.tile([1, F], DT)
            tmpL = spool.tile([1, L], DT)

            def fold(dst, src_row):
                nc.vector.tensor_reduce(
                    dst[:], src_row.rearrange("o (dx l) -> o l dx", l=L),
                    axis=mybir.AxisListType.X, op=AL.add)

            fold(cnt, row_c[:])
            fold(sy, row_y[:])
            fold(sx, row_x[:])  # = sum of xb*onehot
            nc.vector.tensor_tensor(tmpF[:], row_c[:], dxpat[:], AL.mult)
            fold(tmpL, tmpF[0:1, :])
            nc.vector.tensor_tensor(sx[:], sx[:], tmpL[:], AL.add)

            # ---- centroids: cy = floor(sy/max(cnt,1)), cx likewise ----
            cnts = spool.tile([1, L], DT)
            nc.vector.tensor_scalar_max(cnts[:], cnt[:], 1.0)
            rec = spool.tile([1, L], DT)
            nc.vector.reciprocal(rec[:], cnts[:])

            def floordiv(dst, num):
                # q0 = num * (1/cnt) rounded to int (any mode, within +-1 of
                # floor); the two correction steps repair to exact floor.
                q = spool.tile([1, L], DT, tag="fd_q")
                f = spool.tile([1, L], DT, tag="fd_f")
                qi = spool.tile([1, L], mybir.dt.int32, tag="fd_i")
                nc.vector.tensor_tensor(q[:], num[:], rec[:], AL.mult)
                nc.vector.tensor_copy(qi[:], q[:])
                nc.vector.tensor_copy(q[:], qi[:])
                nc.vector.tensor_tensor(f[:], q[:], cnts[:], AL.mult)
                nc.vector.tensor_tensor(f[:], f[:], num[:], AL.is_gt)
                nc.vector.tensor_tensor(q[:], q[:], f[:], AL.subtract)
                nc.vector.tensor_scalar_add(f[:], q[:], 1.0)
                nc.vector.tensor_tensor(f[:], f[:], cnts[:], AL.mult)
                nc.vector.tensor_tensor(f[:], f[:], num[:], AL.is_le)
                nc.vector.tensor_tensor(dst[:], q[:], f[:], AL.add)

            cy = spool.tile([1, L], DT)
            cx = spool.tile([1, L], DT)
            floordiv(cy, sy)
            floordiv(cx, sx)

            # valid = (cnt > 0) & (l > 0); bigm = BIG where invalid else 0
            valid = spool.tile([1, L], DT)
            bigm = spool.tile([1, L], DT)
            nc.vector.tensor_scalar(valid[:], cnt[:], 0.5, None, op0=AL.is_gt)
            nc.vector.tensor_tensor(valid[:], valid[:], lmask[:], AL.mult)
            nc.vector.tensor_scalar(bigm[:], valid[:], -float(BIG), float(BIG),
                                    op0=AL.mult, op1=AL.add)

            # ---- Phase 1.5: build R [2, W*L] ----
            # row0[(x,l)] = -2*cy_l ; row1[(x,l)] = (x-cx_l)^2 + cy_l^2 + bigm_l
            r_ones = spool.tile([1, L], DT)
            r_m2cx = spool.tile([1, L], DT)
            r_quad = spool.tile([1, L], DT)
            nc.vector.memset(r_ones[:], 1.0)
            nc.vector.tensor_scalar_mul(r_m2cx[:], cx[:], -2.0)
            nc.vector.tensor_tensor(r_quad[:], cx[:], cx[:], AL.mult)
            nc.vector.tensor_tensor(tmpL[:], cy[:], cy[:], AL.mult)
            nc.vector.tensor_tensor(r_quad[:], r_quad[:], tmpL[:], AL.add)
            nc.vector.tensor_tensor(r_quad[:], r_quad[:], bigm[:], AL.add)
            rhs3 = spool.tile([3, L], DT)
            nc.sync.dma_start(rhs3[0:1, :], r_ones[:])
            nc.sync.dma_start(rhs3[1:2, :], r_m2cx[:])
            nc.sync.dma_start(rhs3[2:3, :], r_quad[:])
            m2cy = spool.tile([1, L], DT)
            nc.vector.tensor_scalar_mul(m2cy[:], cy[:], -2.0)

            ones128 = spool.tile([1, 128], DT)
            nc.vector.memset(ones128[:], 1.0)
            psumT = ps1.tile([128, 5 * L], DT)
            sT = spool.tile([128, 5 * L], DT)
            nc.tensor.matmul(psumT[:, 0:L], ones128[:], m2cy[:],
                             start=True, stop=True)
            for c in range(4):
                nc.tensor.matmul(
                    psumT[:, (c + 1) * L : (c + 2) * L],
                    T1lhs[:, 128 * c : 128 * (c + 1)], rhs3[:],
                    start=True, stop=True)
            nc.vector.tensor_copy(sT[:], psumT[:])
            nc.sync.dma_start(rs0_d.ap(), sT[:, 0:L])
            for c in range(4):
                nc.sync.dma_start(rs1_d.ap()[c], sT[:, (c + 1) * L : (c + 2) * L])

            R = cpool.tile([2, W * L], DT)
            nc.sync.dma_start(
                R[1:2, :],
                rs1_d.ap().rearrange("c p l -> (c p l)").unsqueeze(0))
            rs0_flat = rs0_d.ap().rearrange("p l -> (p l)").unsqueeze(0)
            for c in range(4):
                nc.sync.dma_start(R[0:1, 8192 * c : 8192 * (c + 1)], rs0_flat)

            # ---- Phase 2: min-distance field, heat, squared error ----
            acc4 = spool.tile([128, NYC], DT)
            for yc in range(NYC):
                Gacc = wpool.tile([128, W], DT, tag="gacc")
                pred = wpool.tile([128, W], DT, tag="pred")
                nc.sync.dma_start(pred[:], pred_d.ap()[128 * yc : 128 * (yc + 1), :])
                for xc in range(NXC):
                    psum2 = ps2.tile([128, F], DT, tag="p2")
                    nc.tensor.matmul(
                        psum2[:], Y2[:, 128 * yc : 128 * (yc + 1)],
                        R[:, F * xc : F * (xc + 1)], start=True, stop=True)
                    nc.vector.tensor_reduce(
                        Gacc[:, 8 * xc : 8 * (xc + 1)],
                        psum2[:].rearrange("p (dx l) -> p dx l", l=L),
                        axis=mybir.AxisListType.X, op=AL.min)
                heat = wpool.tile([128, W], DT, tag="heat")
                nc.scalar.activation(
                    heat[:], Gacc[:], mybir.ActivationFunctionType.Exp,
                    bias=negy2[:, yc : yc + 1],
                    scale=-1.0 / (2.0 * SIGMA * SIGMA))
                nc.vector.tensor_tensor(heat[:], heat[:], pred[:], AL.subtract)
                nc.vector.tensor_tensor(heat[:], heat[:], heat[:], AL.mult)
                nc.vector.tensor_reduce(
                    acc4[:, yc : yc + 1], heat[:],
                    axis=mybir.AxisListType.X, op=AL.add)

            # ---- partial sum over partitions -> scalar ----
            acc = spool.tile([128, 1], DT)
            nc.vector.tensor_reduce(
                acc[:], acc4[:], axis=mybir.AxisListType.X, op=AL.add)
            psumF = ps1.tile([1, 1], DT)
            nc.tensor.matmul(psumF[:], acc[:], selAll[:, 0:1], start=True, stop=True)
            outS = spool.tile([1, 1], DT)
            nc.vector.tensor_copy(outS[:], psumF[:])
            nc.sync.dma_start(out_d.ap(), outS[:])


_CACHE = {}


def _get_compiled():
    if "nc" not in _CACHE:
        nc = bacc.Bacc("TRN2", target_bir_lowering=False, debug=False,
                       num_devices=NCORES)
        build(nc)
        nc.compile()
        _CACHE["nc"] = nc
    return _CACHE["nc"]


def kernel(pred_centroids, instance_masks, num_labels):
    pred_centroids = np.asarray(pred_centroids)
    instance_masks = np.asarray(instance_masks)
    assert pred_centroids.shape == (NCORES, 1, H, W)
    assert instance_masks.shape == (NCORES, H, W)
    assert int(num_labels) == L

    nc = _get_compiled()
    consts = _constants()
    in_maps = []
    for c in range(NCORES):
        m = dict(consts)
        m["lab"] = np.ascontiguousarray(instance_masks[c]).astype(np.int32)
        m["pred"] = np.ascontiguousarray(
            pred_centroids[c, 0]).astype(np.float32)
        in_maps.append(m)

    res = run_bass_kernel_spmd(nc, in_maps, list(range(NCORES)))
    total = np.float64(0.0)
    for c in range(NCORES):
        total += np.float64(res.results[c]["out"][0, 0])
    return np.float32(total / (NCORES * H * W))
